# revision 5
# baseline (speedup 1.0000x reference)
"""Trainium2 Bass kernel for nn_Attention_72791105732908 (sparse_attention).

Reference computation (L=2048, B=64, H=1024, HC=1024):
    outs   = prev_layer_outputs.transpose(1, 0, 2)              # [B, L, H]
    energy = tanh(concat([hidden_bcast, outs], -1) @ W_e.T + b_e)  # [B, L, HC]
    attn   = energy @ W_v                                        # [B, L]
    attn   = where(mask == 0, -1e10, attn); softmax over L
    out    = einsum('bl,blh->bh', attn, outs)[None]              # [1, B, H]

Strategy:
  - Data-parallel over batch: core i handles batches 8i..8i+7. No collectives.
  - q[b] = hidden[b] @ W_h.T + b_e is computed on the host (tiny) and shipped
    as the tanh bias; the device only runs the big matmul outs @ W_o.T.
  - The big matmul runs in fp8e4 with perf_mode=DoubleRow (2 fp8 contraction
    elements per partition per cycle): K=1024 in 4 passes of 256. Host ships
    W_o pre-scaled by 64 in fp8; activations are cast bf16->fp8 on-chip (the
    DMA transpose cannot write 1-byte elements, and the DoubleRow pair stride
    must be >=16B, so a host-side fp8 pack cannot be transposed directly).
    tanh(pse/64 + q) un-scales via the activation's scale operand.
  - outs arrives [L, b, H]; the energy matmul contracts over H, so outs is
    transposed to [H, L] tiles with the DMA xbar (2-byte dtype, DRAM->SBUF,
    mapping T[p, j, l] = outs[l, 128j + p]). The [P, JH, LCH] tile layout is
    exactly the DoubleRow operand layout: pairs = adjacent j-subtiles.
  - tanh writes fp8 energy tiles [P, MC, LCH]; the score matvec is 4 DoubleRow
    MMs over m-block pairs with host-scaled W_v (x1024), plus one K=1 fp8
    matmul folding the mask in as (-240)*(240*mask01) = -57600 per masked
    element, so exp((score*1024 - 57600)/1024) = exp(score - 56) == 0.
  - Exp runs on ACT with accum_out producing the chunk softmax denominator
    for free (no DVE masking/reduction work at all).
  - The weighted sum over L runs on the vector engine as ONE broadcast
    tensor_mul + ONE 3-D reduce_sum per 512-chunk, on bf16 transposed tiles
    (fp8 would cost ~3.6% output error; bf16 keeps it at ~0.2%). The masked
    weights are broadcast to all partitions by a K=1 ones matmul.
  - All cross-engine consumers of PE results are deferred on the PE queue so
    the PE never head-of-line blocks; fp8 casts for batch b+1 are deferred
    into batch b's slots (their DMAs are issued one batch ahead).
"""
import numpy as np
import ml_dtypes

import concourse.bacc as bacc
import concourse.mybir as mybir
import concourse.tile as tile
from concourse.bass_utils import run_bass_kernel_spmd
from concourse.masks import make_identity

dt = mybir.dt
AF = mybir.ActivationFunctionType
PM = mybir.MatmulPerfMode

L, B, H, HC = 2048, 64, 1024, 1024
NCORES = 8
BPC = B // NCORES        # batches per core
P = 128
JH = H // P              # 8 h-subtiles (contraction)
JQ = JH // 2             # 4 DoubleRow pair-passes
MC = HC // P             # 8 c-blocks
MQ = MC // 2             # 4 score pair-passes
L4 = L // 512            # 4 chunks of 512 along L
LCH = 512                # l-chunk width
NCH = LCH // P           # 4 transpose dmas per l4 tile

_CACHE = {}
BF = ml_dtypes.bfloat16
F8 = ml_dtypes.float8_e4m3   # TRN fp8e4: max +-240
WO_SCALE = 64.0              # |W_o| <= 0.0221 -> <= 1.41
WV_SCALE = 1024.0            # |W_v| <= 1/32   -> <= 32
CHUNK_DEFER = 3   # energy-block slots between a chunk's exp and its DVE work
END_DEFER = 5     # slots between the last chunk and the batch epilogue
TB_BUFS = 2 * L4  # bf16 transpose-tile prefetch depth (2 batches)
T8_BUFS = 2 * L4  # fp8 cast-tile depth (2 batches)
PSE_BUFS = 3      # energy psum buffering
ET_BUFS = 3
SM_BUFS = 2
CH_BUFS = 3


def _build():
    nc = bacc.Bacc()
    prev = nc.dram_tensor("prev", [L, BPC, H], dt.bfloat16, kind="ExternalInput")
    WoT8 = nc.dram_tensor("WoT8", [P, JH, HC], dt.float8e4, kind="ExternalInput")
    WvT8 = nc.dram_tensor("WvT8", [P, MC, 16], dt.float8e4, kind="ExternalInput")
    qbT = nc.dram_tensor("qbT", [P, MC, BPC], dt.float32, kind="ExternalInput")
    m8 = nc.dram_tensor("m8", [BPC, L], dt.float8e4, kind="ExternalInput")
    out = nc.dram_tensor("out", [BPC, JH, P], dt.float32, kind="ExternalOutput")

    with tile.TileContext(nc) as tc:
        with (
            tc.tile_pool(name="const", bufs=1) as const,
            tc.tile_pool(name="data", bufs=TB_BUFS) as data,
            tc.tile_pool(name="dat8", bufs=T8_BUFS) as dat8,
            tc.tile_pool(name="et", bufs=ET_BUFS) as etp,
            tc.tile_pool(name="small", bufs=SM_BUFS) as small,
            tc.tile_pool(name="chnk", bufs=CH_BUFS) as chnk,
            tc.tile_pool(name="pse", bufs=PSE_BUFS, space="PSUM") as pse_p,
            tc.tile_pool(name="pss", bufs=2, space="PSUM") as pss_p,
            tc.tile_pool(name="psr", bufs=1, space="PSUM") as psr_p,
            tc.tile_pool(name="pso", bufs=1, space="PSUM") as pso_p,
        ):
            # ---- constants on the ACT HWDGE ring (don't queue behind the
            # activation transposes on the SP ring)
            wo8 = const.tile([P, JH, HC], dt.float8e4)
            nc.scalar.dma_start(out=wo8[:], in_=WoT8[:])
            wv8 = const.tile([P, MC, 16], dt.float8e4)
            nc.scalar.dma_start(out=wv8[:], in_=WvT8[:])
            qb = const.tile([P, MC, BPC], dt.float32)
            nc.scalar.dma_start(out=qb[:], in_=qbT[:])
            neg240 = const.tile([1, 1], dt.float8e4)
            nc.vector.memset(neg240[:], -240.0)
            ones_bf = const.tile([1, P], dt.bfloat16)
            nc.vector.memset(ones_bf[:], 1.0)
            ones_f = const.tile([1, P], dt.float32)
            nc.vector.memset(ones_f[:], 1.0)
            ident = const.tile([P, P], dt.float32)
            make_identity(nc, ident[:])

            # ---- deferred-emission scheduler over energy-block slots.
            # Global block index g = (b*L4 + l4)*MC + m; sched[g] holds thunks
            # emitted right after energy block g.
            sched = {}
            NBLK = BPC * L4 * MC

            def defer(g, thunk):
                if g >= NBLK:
                    sched.setdefault(NBLK, []).append(thunk)
                else:
                    sched.setdefault(g, []).append(thunk)

            def make_cast(tb4, tb8):
                def cast():
                    nc.scalar.activation(tb8[:], tb4[:], AF.Copy)
                return cast

            def make_score(pss, wv8, etm, q):
                def score():
                    nc.tensor.matmul(
                        pss[:], wv8[:, 2 * q:2 * q + 2, 0:1],
                        etm[:, 2 * q:2 * q + 2, :],
                        start=(q == 0), stop=False, perf_mode=PM.DoubleRow,
                    )
                return score

            def make_mask_exp(pss, m01, s4, l4, wnb):
                def mask_exp():
                    nc.tensor.matmul(
                        pss[:], neg240[:],
                        m01[0:1, l4 * LCH:(l4 + 1) * LCH],
                        start=False, stop=True,
                    )
                    nc.scalar.activation(wnb[:], pss[:], AF.Exp,
                                         scale=1.0 / WV_SCALE,
                                         accum_out=s4[0:1, l4:l4 + 1])
                return mask_exp

            def make_chunk(tb4, wnb, wsum4, l4):
                """Broadcast weights + partial weighted sum for one chunk."""
                def chunk():
                    # broadcast weights to all partitions (K=1 ones matmul)
                    psr = psr_p.tile([P, LCH], dt.float32, tag="psr")
                    nc.tensor.matmul(psr[:], ones_bf[:], wnb[:],
                                     start=True, stop=True)
                    wrep = chnk.tile([P, LCH], dt.bfloat16, tag="wrep")
                    nc.scalar.activation(wrep[:], psr[:], AF.Copy)
                    # wsum4[p, j, l4] = sum_l tb4[p, j, l] * wrep[p, l] on DVE
                    junk = chnk.tile([P, JH, LCH], dt.bfloat16, tag="junk")
                    nc.vector.tensor_mul(
                        junk[:], tb4[:],
                        wrep[:].unsqueeze(1).broadcast_to([P, JH, LCH]))
                    nc.vector.reduce_sum(wsum4[:, :, l4:l4 + 1], junk[:],
                                         axis=mybir.AxisListType.X)
                return chunk

            def make_end(b, wsum4, s4):
                def end():
                    ssum = small.tile([1, 1], dt.float32, tag="ssum")
                    nc.vector.reduce_sum(ssum[:], s4[:], axis=mybir.AxisListType.X)
                    wsum = small.tile([P, JH], dt.float32, tag="wsum")
                    nc.vector.reduce_sum(wsum[:].unsqueeze(2), wsum4[:],
                                         axis=mybir.AxisListType.X)
                    rsum = small.tile([1, 1], dt.float32, tag="rsum")
                    nc.vector.reciprocal(rsum[:], ssum[:])
                    # broadcast 1/sum to 128 partitions (K=1 matmul)
                    psb = pso_p.tile([P, JH], dt.float32, tag="pso")
                    nc.tensor.matmul(psb[:, 0:1], ones_f[:], rsum[:],
                                     start=True, stop=True)
                    rsp = small.tile([P, 1], dt.float32, tag="rsp")
                    nc.vector.tensor_copy(rsp[:], psb[:, 0:1])
                    wfin = small.tile([P, JH], dt.float32, tag="wfin")
                    nc.vector.tensor_scalar_mul(wfin[:], wsum[:], rsp[:])
                    # transpose [128, 8] -> [8, 128] and write out
                    pst = pso_p.tile([JH, P], dt.float32, tag="pso")
                    nc.tensor.transpose(pst[:], wfin[:], ident[:])
                    ob = small.tile([JH, P], dt.float32, tag="ob")
                    nc.vector.tensor_copy(ob[:], pst[:])
                    nc.sync.dma_start(out=out[b], in_=ob[:])
                return end

            def emit_loads(b):
                """Transpose-DMA batch b's tiles; return (tb4s, tb8s, m01)."""
                tb4s, tb8s = [], []
                for l4 in range(L4):
                    tb4 = data.tile([P, JH, LCH], dt.bfloat16, tag="tb")
                    for cc in range(NCH):
                        c = l4 * NCH + cc
                        nc.sync.dma_start(
                            out=tb4[:, :, cc * P:(cc + 1) * P],
                            in_=prev[c * P:(c + 1) * P, b, :],
                            transpose=True,
                        )
                    tb4s.append(tb4)
                    tb8s.append(dat8.tile([P, JH, LCH], dt.float8e4,
                                          name="tb8", tag="t8"))
                m01 = small.tile([1, L], dt.float8e4, tag="m01")
                nc.sync.dma_start(out=m01[:], in_=m8[b:b + 1, :])
                return tb4s, tb8s, m01

            # ---- main emission loop
            batch_tiles = {0: emit_loads(0)}
            for l4 in range(L4):  # batch 0 casts run up front
                nc.scalar.activation(batch_tiles[0][1][l4][:],
                                     batch_tiles[0][0][l4][:], AF.Copy)

            for b in range(BPC):
                tb4s, tb8s, m01 = batch_tiles.pop(b)
                base_b = b * L4 * MC
                if b + 1 < BPC:
                    # issue b+1's DMAs now; cast fp8 tiles in deferred slots
                    batch_tiles[b + 1] = emit_loads(b + 1)
                    for l4 in range(L4):
                        defer(base_b + (l4 + 1) * MC - 2,
                              make_cast(batch_tiles[b + 1][0][l4],
                                        batch_tiles[b + 1][1][l4]))

                wsum4 = small.tile([P, JH, L4], dt.float32, tag="wsum4")
                s4 = small.tile([1, L4], dt.float32, tag="s4")

                for l4 in range(L4):
                    tb4, tb8 = tb4s[l4], tb8s[l4]
                    etm = etp.tile([P, MC, LCH], dt.float8e4, tag="etm")
                    pss = pss_p.tile([1, LCH], dt.float32, tag="pss")
                    for m in range(MC):
                        g = (b * L4 + l4) * MC + m
                        pse = pse_p.tile([P, LCH], dt.float32, tag="pse")
                        for jq in range(JQ):
                            nc.tensor.matmul(
                                pse[:],
                                wo8[:, 2 * jq:2 * jq + 2, m * P:(m + 1) * P],
                                tb8[:, 2 * jq:2 * jq + 2, :],
                                start=(jq == 0), stop=(jq == JQ - 1),
                                perf_mode=PM.DoubleRow,
                            )
                        for thunk in sched.pop(g, []):
                            thunk()
                        nc.scalar.activation(etm[:, m, :], pse[:], AF.Tanh,
                                             bias=qb[:, m, b:b + 1],
                                             scale=1.0 / WO_SCALE)
                        if m % 2 == 1:
                            defer(g + 1, make_score(pss, wv8, etm, m // 2))
                        if m == MC - 1:
                            wnb = chnk.tile([1, LCH], dt.bfloat16, tag="wnb")
                            defer(g + 2, make_mask_exp(pss, m01, s4, l4, wnb))
                            defer(g + CHUNK_DEFER,
                                  make_chunk(tb4, wnb, wsum4, l4))
                            if l4 == L4 - 1:
                                defer(g + END_DEFER, make_end(b, wsum4, s4))

            for g in sorted(sched):
                for thunk in sched[g]:
                    thunk()

    nc.finalize()
    return nc


def _in_maps(prev_layer_outputs, hidden, mask, W_e, b_e, W_v):
    # host-side layout prep (cheap, O(MB) except the bf16 cast of prev)
    Wo = W_e[:, H:]
    WoT8 = np.ascontiguousarray(
        (WO_SCALE * Wo).T.reshape(JH, P, HC).transpose(1, 0, 2)).astype(F8)
    WvT8 = np.zeros((P, MC, 16), dtype=F8)
    WvT8[:, :, 0] = (WV_SCALE * W_v).reshape(MC, P).T.astype(F8)
    q_full = (hidden.astype(np.float32) @ W_e[:, :H].astype(np.float32).T
              + b_e.astype(np.float32))                       # [B, HC]
    m8_full = np.where(mask == 0, 240.0, 0.0).astype(F8)

    def _shard(i):
        bs = slice(i * BPC, (i + 1) * BPC)
        prev_i = prev_layer_outputs[:, bs, :].astype(BF)
        qbT_i = np.ascontiguousarray(
            q_full[bs].T.reshape(MC, P, BPC).transpose(1, 0, 2)
        ).astype(np.float32)
        return {
            "prev": prev_i, "WoT8": WoT8, "WvT8": WvT8,
            "qbT": qbT_i, "m8": np.ascontiguousarray(m8_full[bs]),
        }

    from concurrent.futures import ThreadPoolExecutor
    with ThreadPoolExecutor(NCORES) as ex:
        in_maps = list(ex.map(_shard, range(NCORES)))
    return in_maps


def kernel(prev_layer_outputs, hidden, mask, W_e, b_e, W_v):
    prev_layer_outputs = np.asarray(prev_layer_outputs)
    hidden = np.asarray(hidden)
    mask = np.asarray(mask)
    W_e = np.asarray(W_e)
    b_e = np.asarray(b_e)
    W_v = np.asarray(W_v)
    if "nc" not in _CACHE:
        _CACHE["nc"] = _build()
    nc = _CACHE["nc"]
    in_maps = _in_maps(prev_layer_outputs, hidden, mask, W_e, b_e, W_v)
    res = run_bass_kernel_spmd(nc, in_maps, list(range(NCORES)))
    out = np.concatenate(
        [np.asarray(r["out"]).reshape(1, BPC, H) for r in res.results], axis=1)
    return out.astype(np.float32)


def run_traced(inputs):
    """Profiled run (test harness only)."""
    if "nc" not in _CACHE:
        _CACHE["nc"] = _build()
    nc = _CACHE["nc"]
    in_maps = _in_maps(**inputs)
    return run_bass_kernel_spmd(nc, in_maps, list(range(NCORES)), trace=True)
